# revision 6
# baseline (speedup 1.0000x reference)
"""Trainium2 Bass kernel for nn_Cycle_Consistency_Loss (soft-DTW-style
cycle loss), SPMD over 8 NeuronCores.  748us baseline -> 431us.

Math (per pair (a,b), both directions; x = seq[q], y = seq[k], lens = src_len//4):
  alpha = softmax_j(-|x_i-y_j|^2) over valid j -> snn = alpha @ y
  beta  = softmax_k(-|snn_i-x_k|^2) over valid k
  u = E_beta[k], std = E_beta[(k-u)^2]
  li = (i-u)^2/std + 0.005*ln(std), summed over valid i; total / n_pairs.

Work items = 512-query blocks of each direction, packed into NS steps x
8 cores (greedy + local search minimizing sum of per-step group maxes).
Scores are computed transposed ([keys->partitions, queries->free]) via
augmented fp16 matmuls; softmax weights exp'd on ACT ([128,1024] per
2-chunk group, bf16 out); snn numerator / per-chunk index moments
accumulate on the PE.  The kernel is ACT-throughput-bound (~365us of
exp) with the PE a close second; the structure keeps both >80% busy:

  - Concurrent score-MM pairs: contract is only 36 (34 for pass B) rows,
    so each group's two key-chunk matmuls are placed in disjoint 64-row
    strips (partitions 0:36 / 64:100 of kA/kB/qA/R2) and run CONCURRENTLY
    on the PE's row-tiled subarrays, halving score-MM wall time and
    hiding their LDWEIGHTS (keeps HAM warm).  Value-side matmuls can't
    pair (col-tiling needs a shared moving operand).
  - Software pipeline: pass B1 of step s-1 is emitted between pass A of
    step s and the epilogues, so the R2 chain (Z -> ln -> exp -> gpsimd
    broadcast -> mul) has a full pass of slack; B2's reduce matmul +
    stats store are deferred one further iteration.
  - ACT table thrash fix: Exp/Ln both served from the combined
    natural_log_exp_and_others set (claim-set steering at build time):
    28 alternating ACT_TABLE_LOADs -> 1.
  - 1/Z = exp(-ln Z) on ACT (shortest chain); 1/Z2 via 4+4 DMA-transpose
    to [128,4] + DVE reciprocal; final-phase 1/std via exp(-ln std).
  - Stats stored as rows of [64, 512] tiles (last step at partition 32
    so the tail's final-row math has a legal 32-aligned base); rows
    0..NS-2 of the final loss math are emitted before the flush.

Precision (rel err ~2e-3 vs fp32 ref, gate 2e-2): fp16 score operands
with hi/lo-split squared norms; bf16 softmax weights; per-chunk moment
recombination in fp32 with 128-aligned index offsets (cancellation
~1e-3 -> STD_FLOOR 2e-3).
"""
import sys
import numpy as np

sys.path.insert(0, "/opt/trn_rl_repo")

QB = 512          # query block = matmul free dim = one PSUM bank of fp32
KG = 256          # key group (2 chunks of 128 partitions)
NCORES = 8
PENALTY = 0.01
BIGM = 60000.0    # fp16-representable mask value
STD_FLOOR = 2.0e-3

INPUT_NAMES = ("kA", "vAr", "qA", "kB", "kmom", "cst", "qidx", "qmask")


def _ceil(a, b):
    return -(-a // b)


class _Item:
    __slots__ = ("qi", "ki", "Lq", "Lk", "qb", "ga", "gb", "dummy")

    def __init__(self, qi, ki, Lq, Lk, qb):
        self.qi, self.ki, self.Lq, self.Lk, self.qb = qi, ki, Lq, Lk, qb
        self.ga = _ceil(Lk, KG)
        self.gb = _ceil(Lq, KG)
        self.dummy = False


class _Dummy:
    qi = ki = Lq = Lk = qb = 0
    ga = gb = 0
    dummy = True


def _steps_cost(steps):
    return (sum(max((x.ga for x in s), default=0) for s in steps)
            + sum(max((x.gb for x in s), default=0) for s in steps))


def _pack_steps(items, ns):
    """Try several greedy weights + local search + random restarts; keep
    the cheapest (ACT cost is equal per A/B group: minimize sum of
    per-step maxes)."""
    import random
    rng = random.Random(1234)
    best, bcost = None, None
    seeds = [sorted(items, key=lambda t: -(t.ga + cb * t.gb))
             for cb in (1.0, 1.3, 1.6)]
    for r in range(6):
        seeds.append(rng.sample(items, len(items)))
    for order in seeds:
        steps = _pack_steps_w(list(order), ns, 1.0, pre_sorted=True)
        cost = _steps_cost(steps)
        if bcost is None or cost < bcost:
            bcost, best = cost, steps
    return best


def _pack_steps_w(items, ns, CB, pre_sorted=False):
    steps = [[] for _ in range(ns)]
    order = items if pre_sorted else sorted(
        items, key=lambda t: -(t.ga + CB * t.gb))
    for it in order:
        best, bcost = None, None
        for s in range(ns):
            if len(steps[s]) >= NCORES:
                continue
            ga_m = max((x.ga for x in steps[s]), default=0)
            gb_m = max((x.gb for x in steps[s]), default=0)
            cost = (max(ga_m, it.ga) - ga_m + CB * (max(gb_m, it.gb) - gb_m))
            key = (cost, -len(steps[s]))
            if bcost is None or key < bcost:
                bcost, best = key, s
        steps[best].append(it)

    def _cost():
        return (sum(max((x.ga for x in s), default=0) for s in steps)
                + CB * sum(max((x.gb for x in s), default=0) for s in steps))

    base = _cost()
    for _ in range(200):
        improved = False
        for si in range(ns):
            for sj in range(ns):
                if si == sj:
                    continue
                for i in range(len(steps[si])):
                    if len(steps[sj]) < NCORES:
                        it = steps[si].pop(i)
                        steps[sj].append(it)
                        c = _cost()
                        if c < base - 1e-9:
                            base = c
                            improved = True
                            break
                        steps[sj].pop()
                        steps[si].insert(i, it)
                    done = False
                    for j in range(len(steps[sj])):
                        steps[si][i], steps[sj][j] = steps[sj][j], steps[si][i]
                        c = _cost()
                        if c < base - 1e-9:
                            base = c
                            improved = done = True
                            break
                        steps[si][i], steps[sj][j] = steps[sj][j], steps[si][i]
                    if done:
                        break
                else:
                    continue
                break
        if not improved:
            break
    for s in range(ns):
        while len(steps[s]) < NCORES:
            steps[s].append(_Dummy())
    return steps


def _split_hi_lo16(v):
    hi = v.astype(np.float16).astype(np.float32)
    lo = (v - hi).astype(np.float16).astype(np.float32)
    return hi, lo


def _bf16r(a):
    b = np.asarray(a, np.float32).copy()
    v = b.view(np.uint32)
    v += 0x8000
    v &= 0xFFFF0000
    return b


def pack(seq, src_len, combinations):
    """Build the step plan and per-core input arrays.

    Per-core inputs:
      kA   fp16 [36, CA]    pass-A keys [y; y2h; y2l; 1; 1] (masked y2h=BIGM)
      vAr  fp16 [128, CA//128*33]  pass-A values pre-swizzled:
                            vAr[p, c*33+d] = vA[c*128+p, d], vA = [y | 1]
      qA   fp16 [36, QB*NS] pass-A queries [2xT; -1; -1; -x2h; -x2l]
      kB   fp16 [34, CB]    pass-B keys [2xT; x2h; x2l] (masked x2h=BIGM)
      kmom bf16 [128, 32*128] chunk-moment lhsT: for chunk ch, col ch =
                            1, col 32+ch = -2p, col 64+ch = 128*(p*p>>7),
                            col 96+ch = p*p&127; zeros elsewhere
      cst  fp32 [64, 35]    col 0-32: redW (Z2/r0 reduce lhsT),
                            col 33 rows 0-31: -128*ch, col 34 rows 0-31: 1
      qidx/qmask fp32 [NS, QB] absolute query index / valid mask per step
    """
    seq = np.asarray(seq, np.float32)
    lens = (np.asarray(src_len).astype(np.int64) // 4).astype(np.int64)
    comb = np.asarray(combinations).astype(np.int64)

    items = []
    for a, b in comb:
        for qi, ki in ((a, b), (b, a)):
            Lq, Lk = int(lens[qi]), int(lens[ki])
            if Lq <= 0 or Lk <= 0:
                continue
            for qb in range(_ceil(Lq, QB)):
                items.append(_Item(int(qi), int(ki), Lq, Lk, qb))
    NS = max(1, _ceil(len(items), NCORES))
    steps = _pack_steps(items, NS)

    GA = [max(max(it.ga for it in steps[s]), 1) for s in range(NS)]
    GB = [max(max(it.gb for it in steps[s]), 1) for s in range(NS)]
    CA = sum(GA) * KG
    CB = sum(GB) * KG

    sq2 = np.einsum("btd,btd->bt", seq, seq).astype(np.float32)

    p = np.arange(128, dtype=np.float32)
    kmom = np.zeros((128, 32 * 128), np.float32)
    for ch in range(32):
        kmom[:, 128 * ch + ch] = 1.0
        kmom[:, 128 * ch + 32 + ch] = -2.0 * p
        kmom[:, 128 * ch + 64 + ch] = 128.0 * np.floor(p * p / 128)
        kmom[:, 128 * ch + 96 + ch] = p * p - 128.0 * np.floor(p * p / 128)
    cst = np.zeros((64, 35), np.float32)
    cst[0:32, 0] = 1.0                       # Z2 = sum Z2c
    cst[0:32, 32] = 128.0 * np.arange(32)    # r0 += 128*ch*Z2c
    cst[32:64, 32] = -0.5                    # r0 += -0.5 * (-2 sum P2 p)
    cst[0:32, 33] = -128.0 * np.arange(32)   # koff
    cst[0:32, 34] = 1.0                      # ones32 (stdsum reduce lhsT)

    cores = []
    for c in range(NCORES):
        kA = np.zeros((36, CA), np.float32)
        vA = np.zeros((CA, 33), np.float32)
        qA = np.zeros((128, QB * NS), np.float32)
        kB = np.zeros((34, CB), np.float32)
        qidx = np.zeros((64, QB), np.float32)
        qmask = np.zeros((64, QB), np.float32)
        offa = 0
        offb = 0
        its = []
        for s in range(NS):
            it = steps[s][c]
            its.append(it)
            na = GA[s] * KG
            nb = GB[s] * KG
            ka = kA[:, offa:offa + na]
            va = vA[offa:offa + na]
            kb = kB[:, offb:offb + nb]
            qa = qA[:, s * QB:(s + 1) * QB]
            if it.dummy:
                # scores 0 -> P = 1; Z = na via vA flag column
                va[:, 32] = 1.0
            else:
                y = seq[it.ki]
                x = seq[it.qi]
                Lk, Lq = it.Lk, it.Lq
                nk = min(Lk, na)
                ka[0:32, :nk] = y[:nk].T
                y2h, y2l = _split_hi_lo16(sq2[it.ki, :nk])
                ka[32, :nk] = y2h
                ka[33, :nk] = y2l
                ka[32, nk:] = BIGM
                ka[34, :nk] = 1.0
                ka[35, :nk] = 1.0
                va[:nk, 0:32] = y[:nk]
                va[:nk, 32] = 1.0
                q0 = it.qb * QB
                nq = min(Lq - q0, QB)
                qa[0:32, :nq] = 2.0 * x[q0:q0 + nq].T
                qa[32, :nq] = -1.0
                qa[33, :nq] = -1.0
                x2h, x2l = _split_hi_lo16(sq2[it.qi, q0:q0 + nq])
                qa[34, :nq] = -x2h
                qa[35, :nq] = -x2l
                nkb = min(Lq, nb)
                kb[0:32, :nkb] = 2.0 * x[:nkb].T
                xh, xl = _split_hi_lo16(sq2[it.qi, :nkb])
                kb[32, :nkb] = xh
                kb[33, :nkb] = xl
                kb[32, nkb:] = BIGM
                ii = q0 + np.arange(QB)
                srow = 32 if s == NS - 1 else s
                qidx[srow, :] = ii.astype(np.float32)
                qmask[srow, :] = (ii < Lq).astype(np.float32)
            offa += na
            offb += nb
        vAr = np.ascontiguousarray(
            vA.reshape(CA // 128, 128, 33).transpose(1, 0, 2).reshape(128, -1))

        # paired-strip layouts: chunk pair g -> strip h=0 at partitions
        # 0:{rows}, strip h=1 at partitions 64:64+{rows}; 128 cols per pair
        qA[64:100, :] = qA[0:36, :]
        kA2 = np.zeros((128, CA // 2), np.float32)
        kr = kA.reshape(36, CA // 256, 2, 128)
        kA2.reshape(128, CA // 256, 128)[0:36] = kr[:, :, 0, :]
        kA2.reshape(128, CA // 256, 128)[64:100] = kr[:, :, 1, :]
        kB2 = np.zeros((128, CB // 2), np.float32)
        kbr = kB.reshape(34, CB // 256, 2, 128)
        kB2.reshape(128, CB // 256, 128)[0:34] = kbr[:, :, 0, :]
        kB2.reshape(128, CB // 256, 128)[64:98] = kbr[:, :, 1, :]

        import ml_dtypes
        cores.append(dict(
            kA=kA2.astype(np.float16),
            vAr=vAr.astype(np.float16),
            qA=qA.astype(np.float16),
            kB=kB2.astype(np.float16),
            kmom=_bf16r(kmom).astype(ml_dtypes.bfloat16),
            cst=cst,
            qidx=qidx, qmask=qmask, items=its))
    plan = dict(NS=NS, GA=GA, GB=GB, CA=CA, CB=CB)
    return plan, cores


def build_program(plan, debug=False):
    """Build the SPMD Bass program for the given step plan."""
    import concourse.bass as bass
    import concourse.bacc as bacc
    import concourse.mybir as mybir
    import concourse.tile as tile
    from concourse.hw_specs import get_activation_tables

    F32 = mybir.dt.float32
    F16 = mybir.dt.float16
    BF16 = mybir.dt.bfloat16
    AFT = mybir.ActivationFunctionType
    NS, GA, GB = plan["NS"], plan["GA"], plan["GB"]
    CA, CB = plan["CA"], plan["CB"]
    GAmax = max(GA)
    GBmax = max(GB)

    nc = bacc.Bacc("TRN2", target_bir_lowering=False, debug=False,
                   num_devices=NCORES)

    # Steer the ACT table-load pass to the combined exp+ln set: drop the
    # claim of Exp/Ln from the exp-only/ln-only sets so the first set
    # serving both is natural_log_exp_and_others (runtime tables for that
    # set genuinely contain both functions).
    tabs = get_activation_tables(nc.m.arch)
    tabs["exp_and_others"].discard(AFT.Exp)
    tabs["natural_log"].discard(AFT.Ln)

    kA_d = nc.dram_tensor("kA", [128, CA // 2], F16, kind="ExternalInput")
    vAr_d = nc.dram_tensor("vAr", [128, (CA // 128) * 33], F16,
                           kind="ExternalInput")
    qA_d = nc.dram_tensor("qA", [128, QB * NS], F16, kind="ExternalInput")
    kB_d = nc.dram_tensor("kB", [128, CB // 2], F16, kind="ExternalInput")
    kmom_d = nc.dram_tensor("kmom", [128, 32 * 128], BF16,
                            kind="ExternalInput")
    cst_d = nc.dram_tensor("cst", [64, 35], F32, kind="ExternalInput")
    qidx_d = nc.dram_tensor("qidx", [64, QB], F32, kind="ExternalInput")
    qmask_d = nc.dram_tensor("qmask", [64, QB], F32, kind="ExternalInput")
    out_d = nc.dram_tensor("out", [1, 1], F32, kind="ExternalOutput")
    if debug:
        dbgSu_d = nc.dram_tensor("dbgSu", [64, QB], F32,
                                 kind="ExternalOutput")
        dbgSs_d = nc.dram_tensor("dbgSs", [64, QB], F32,
                                 kind="ExternalOutput")

    offa_l = np.concatenate([[0], np.cumsum(np.array(GA) * KG)]).tolist()
    offb_l = np.concatenate([[0], np.cumsum(np.array(GB) * KG)]).tolist()

    with tile.TileContext(nc) as tc:
        with (
            tc.tile_pool(name="keys", bufs=2) as keys_pool,
            tc.tile_pool(name="vals", bufs=2) as vals_pool,
            tc.tile_pool(name="qrys", bufs=2) as qrys_pool,
            tc.tile_pool(name="pa", bufs=4) as pa_pool,
            tc.tile_pool(name="epi", bufs=3) as epi_pool,
            tc.tile_pool(name="b2", bufs=3) as b2_pool,
            tc.tile_pool(name="fin", bufs=1) as fin_pool,
            tc.tile_pool(name="sc_ps", bufs=2, space="PSUM") as sc_psum,
            tc.tile_pool(name="na_ps", bufs=1, space="PSUM") as na_psum,
            tc.tile_pool(name="t2_ps", bufs=2, space="PSUM") as t2_psum,
            tc.tile_pool(name="red_ps", bufs=1, space="PSUM") as red_psum,
        ):
            # fin tiles allocated here; their loads are emitted after the
            # step-0 input loads so the first A-pass starts ASAP
            kmom = fin_pool.tile([128, 32 * 128], BF16)
            cst = fin_pool.tile([64, 35], F32)
            qidx = fin_pool.tile([64, QB], F32)
            qmask = fin_pool.tile([64, QB], F32)
            stats_u = fin_pool.tile([64, QB], F32)
            stats_s = fin_pool.tile([64, QB], F32)
            onesNS = fin_pool.tile([64, 1], F32)
            nc.gpsimd.memset(onesNS[:], 1.0)
            # R2 persistent double buffer (both strips): const -1 rows
            # memset once
            R2s = [fin_pool.tile([128, QB], F16, tag=f"R2_{i}",
                                 name=f"R2_{i}")
                   for i in range(2)]
            for i in range(2):
                nc.gpsimd.memset(R2s[i][32:34, :], -1.0)
                nc.gpsimd.memset(R2s[i][96:98, :], -1.0)

            # deferred B2 tail state: (s3, rz2, step_idx)
            tail = None

            for s in range(NS + 1):
                # ---- pass A of step s
                if s < NS:
                    ga = GA[s]
                    na = ga * KG
                    offa = offa_l[s]
                    qA_t = qrys_pool.tile([128, QB], F16, tag="qA")
                    nc.sync.dma_start(qA_t[:], qA_d[:, s * QB:(s + 1) * QB])
                    kA_t = keys_pool.tile([128, GAmax * 128], F16, tag="kA")
                    if s == 0:
                        nc.sync.dma_start(kA_t[:, :128],
                                          kA_d[:, offa // 2:offa // 2 + 128])
                        nc.sync.dma_start(
                            kA_t[:, 128:na // 2],
                            kA_d[:, offa // 2 + 128:(offa + na) // 2])
                    else:
                        nc.sync.dma_start(kA_t[:, :na // 2],
                                          kA_d[:, offa // 2:(offa + na) // 2])
                    vA_t = vals_pool.tile([128, GAmax * 66], F16, tag="vA")
                    nc.sync.dma_start(
                        vA_t[:, :ga * 66],
                        vAr_d[:, (offa // 128) * 33:((offa + na) // 128) * 33])
                    kB_t = keys_pool.tile([128, GBmax * 128], F16, tag="kB")
                    nb_s = GB[s] * KG
                    nc.sync.dma_start(
                        kB_t[:, :nb_s // 2],
                        kB_d[:, offb_l[s] // 2:(offb_l[s] + nb_s) // 2])
                    if s == 0:
                        nc.sync.dma_start(kmom[:], kmom_d[:])
                        nc.sync.dma_start(cst[:], cst_d[:])
                        nc.sync.dma_start(qidx[:], qidx_d[:])
                        nc.sync.dma_start(qmask[:], qmask_d[:])

                    # score MMs (skew-2: snn MMs for group g-2 follow
                    # score MMs for group g so PE never waits on exp)
                    numA = na_psum.tile([33, QB], F32, tag="numA", name="numA")
                    Ps = [None, None, None]

                    def snn_mms(g):
                        P = Ps[g % 3]
                        for h in range(2):
                            nc.tensor.matmul(
                                numA[:],
                                vA_t[:, (2 * g + h) * 33:(2 * g + h + 1) * 33],
                                P[:, h * QB:(h + 1) * QB],
                                start=(g == 0 and h == 0),
                                stop=(g == ga - 1 and h == 1))

                    for g in range(ga):
                        sc = sc_psum.tile([128, 2 * QB], F32, tag="sc")
                        for h in range(2):
                            r0_ = 64 * h
                            nc.tensor.matmul(
                                sc[:, h * QB:(h + 1) * QB],
                                kA_t[r0_:r0_ + 36,
                                     g * 128:(g + 1) * 128],
                                qA_t[r0_:r0_ + 36, :],
                                start=True, stop=True)
                        if g >= 2:
                            snn_mms(g - 2)
                        P = pa_pool.tile([128, 2 * QB], BF16, tag="pa")
                        Ps[g % 3] = P
                        nc.scalar.activation(P[:], sc[:], AFT.Exp)
                    for g in range(max(0, ga - 2), ga):
                        snn_mms(g)

                # ---- pass B1 of step s-1 (uses R2[(s-1)%2], kB of s-1)
                if s >= 1:
                    p_ = s - 1
                    gb = GB[p_]
                    kB_p = kB_ts
                    R2p = R2s[p_ % 2]
                    T2 = t2_psum.tile([128, QB], F32, tag="T2", name="T2")
                    P2s = [None, None, None]

                    def mom_mms(g):
                        P2 = P2s[g % 3]
                        for h in range(2):
                            ch = 2 * g + h
                            nc.tensor.matmul(
                                T2[:],
                                kmom[:, ch * 128:(ch + 1) * 128],
                                P2[:, h * QB:(h + 1) * QB],
                                start=(g == 0 and h == 0),
                                stop=(g == gb - 1 and h == 1))

                    for g in range(gb):
                        sc = sc_psum.tile([128, 2 * QB], F32, tag="sc")
                        for h in range(2):
                            r0_ = 64 * h
                            nc.tensor.matmul(
                                sc[:, h * QB:(h + 1) * QB],
                                kB_p[r0_:r0_ + 34,
                                     g * 128:(g + 1) * 128],
                                R2p[r0_:r0_ + 34, :],
                                start=True, stop=True)
                        if g >= 2:
                            mom_mms(g - 2)
                        P2 = pa_pool.tile([128, 2 * QB], BF16, tag="pb")
                        P2s[g % 3] = P2
                        nc.scalar.activation(P2[:], sc[:], AFT.Exp)
                    for g in range(max(0, gb - 2), gb):
                        mom_mms(g)

                if s < NS:
                    kB_ts = kB_t  # carry this step's kB tile to next iter

                    # ---- epilogue A(s): R2 = [snn/Z; -1; -1] fp16
                    # 1/Z = exp(-ln Z) on ACT: shortest dependency chain;
                    # both funcs served by the combined table set
                    lnz = epi_pool.tile([1, QB], F32, tag="lnz")
                    nc.scalar.activation(lnz[:], numA[32:33, :], AFT.Ln)
                    rz = epi_pool.tile([1, QB], F32, tag="rz")
                    nc.scalar.activation(rz[:], lnz[:], AFT.Exp, scale=-1.0)
                    rb = epi_pool.tile([32, QB], F32, tag="rb")
                    nc.gpsimd.partition_broadcast(rb[:], rz[:])
                    nc.vector.tensor_mul(R2s[s % 2][0:32, :],
                                         numA[0:32, :], rb[:])
                    nc.vector.tensor_copy(R2s[s % 2][64:96, :],
                                          R2s[s % 2][0:32, :])

                # ---- epilogue B1(s-1): Z2/r0 reduce, u = r0/Z2, E chain
                if s >= 1:
                    p_ = s - 1
                    tt2 = epi_pool.tile([64, QB], F32, tag="tt2")
                    nc.vector.tensor_copy(tt2[:], T2[0:64, :])
                    R = red_psum.tile([33, QB], F32, tag="red", name="R")
                    nc.tensor.matmul(R[:], cst[:, 0:33], tt2[:],
                                     start=True, stop=True)
                    z2r = epi_pool.tile([1, QB], F32, tag="z2r")
                    nc.vector.tensor_copy(z2r[:], R[0:1, :])
                    r0row = epi_pool.tile([1, QB], F32, tag="r0row")
                    nc.vector.tensor_copy(r0row[:], R[32:33, :])
                    z2t = epi_pool.tile([128, 4], F32, tag="z2t")
                    for c4 in range(4):
                        nc.sync.dma_start(z2t[:, c4:c4 + 1],
                                          z2r[0:1, c4 * 128:(c4 + 1) * 128])
                    rz2t = epi_pool.tile([128, 4], F32, tag="rz2t")
                    nc.vector.reciprocal(rz2t[:], z2t[:])
                    rz2 = epi_pool.tile([1, QB], F32, tag="rz2")
                    for c4 in range(4):
                        nc.sync.dma_start(rz2[0:1, c4 * 128:(c4 + 1) * 128],
                                          rz2t[:, c4:c4 + 1])
                    u0 = epi_pool.tile([1, QB], F32, tag="u0")
                    nc.vector.tensor_mul(u0[:], r0row[:], rz2[:])
                    prow = 32 if p_ == NS - 1 else p_
                    nc.sync.dma_start(stats_u[prow:prow + 1, :], u0[0:1, :])

                    # B2(s-1): per-chunk variance recombination (DVE)
                    rbU = b2_pool.tile([32, QB], F32, tag="rbU")
                    nc.gpsimd.partition_broadcast(rbU[:], u0[:])
                    E = b2_pool.tile([32, QB], F32, tag="E")
                    nc.vector.tensor_scalar_add(E[:], rbU[:],
                                                cst[0:32, 33:34])
                    E2 = b2_pool.tile([32, QB], F32, tag="E2")
                    nc.vector.tensor_mul(E2[:], E[:], E[:])
                    c1 = b2_pool.tile([32, QB], F32, tag="c1")
                    nc.vector.tensor_mul(c1[:], E2[:], T2[0:32, :])
                    c2 = b2_pool.tile([32, QB], F32, tag="c2")
                    nc.vector.tensor_mul(c2[:], E[:], T2[32:64, :])
                    s1 = b2_pool.tile([32, QB], F32, tag="s1")
                    nc.vector.tensor_add(s1[:], c1[:], c2[:])
                    s2 = b2_pool.tile([32, QB], F32, tag="s2")
                    nc.vector.tensor_add(s2[:], s1[:], T2[64:96, :])
                    s3 = b2_pool.tile([32, QB], F32, tag="s3")
                    nc.vector.tensor_add(s3[:], s2[:], T2[96:128, :])
                    new_tail = (s3, rz2, p_)
                else:
                    new_tail = None

                # ---- B2 tail of step s-2 (deferred: sdout MM + sstd)
                if tail is not None:
                    ps3, prz2, pp = tail
                    sdout = red_psum.tile([33, QB], F32, tag="red", name="sdout")
                    nc.tensor.matmul(sdout[0:1, :], cst[0:32, 34:35], ps3[:],
                                     start=True, stop=True)
                    sstd = epi_pool.tile([1, QB], F32, tag="sstd")
                    nc.vector.tensor_mul(sstd[:], sdout[0:1, :], prz2[:])
                    pr = 32 if pp == NS - 1 else pp
                    nc.sync.dma_start(stats_s[pr:pr + 1, :], sstd[0:1, :])
                tail = new_tail

            # ---- final: li = (i-u)^2/std + 0.005*ln(std), masked sum.
            # Split: rows [0, NS-1) run while the last step's B2 chain
            # resolves; only row NS-1 sits in the tail.
            stdc = fin_pool.tile([64, QB], F32)
            lg = fin_pool.tile([64, QB], F32)
            rstd = fin_pool.tile([64, QB], F32)
            delta = fin_pool.tile([64, QB], F32)
            d2 = fin_pool.tile([64, QB], F32)
            t1 = fin_pool.tile([64, QB], F32)
            lgs = fin_pool.tile([64, QB], F32)
            li = fin_pool.tile([64, QB], F32)
            lim = fin_pool.tile([64, QB], F32)
            nc.gpsimd.memset(lim[:], 0.0)

            def final_rows(r0, r1):
                sl = slice(r0, r1)
                nc.vector.tensor_scalar_max(stdc[sl, :], stats_s[sl, :],
                                            STD_FLOOR)
                nc.scalar.activation(lg[sl, :], stdc[sl, :], AFT.Ln)
                nc.scalar.activation(rstd[sl, :], lg[sl, :], AFT.Exp,
                                     scale=-1.0)
                nc.vector.tensor_sub(delta[sl, :], qidx[sl, :],
                                     stats_u[sl, :])
                nc.vector.tensor_mul(d2[sl, :], delta[sl, :], delta[sl, :])
                nc.vector.tensor_mul(t1[sl, :], d2[sl, :], rstd[sl, :])
                nc.vector.tensor_scalar_mul(lgs[sl, :], lg[sl, :],
                                            0.5 * PENALTY)
                nc.vector.tensor_add(li[sl, :], t1[sl, :], lgs[sl, :])
                nc.vector.tensor_mul(lim[sl, :], li[sl, :], qmask[sl, :])

            if NS > 1:
                final_rows(0, NS - 1)   # rows 0..NS-2

            # flush the last tail
            if tail is not None:
                ps3, prz2, pp = tail
                sdout = red_psum.tile([33, QB], F32, tag="red", name="sdout")
                nc.tensor.matmul(sdout[0:1, :], cst[0:32, 34:35], ps3[:],
                                 start=True, stop=True)
                sstd = epi_pool.tile([1, QB], F32, tag="sstd")
                nc.vector.tensor_mul(sstd[:], sdout[0:1, :], prz2[:])
                pr = 32 if pp == NS - 1 else pp
                nc.sync.dma_start(stats_s[pr:pr + 1, :], sstd[0:1, :])

            if debug:
                nc.sync.dma_start(dbgSu_d[:], stats_u[:])
                nc.sync.dma_start(dbgSs_d[:], stats_s[:])
            final_rows(32, 33)    # last step's row
            rowsum = fin_pool.tile([64, 1], F32)
            nc.vector.reduce_sum(rowsum[:], lim[:],
                                 axis=mybir.AxisListType.X)
            tot = red_psum.tile([33, QB], F32, tag="red", name="tot")
            nc.tensor.matmul(tot[0:1, 0:1], rowsum[:], onesNS[:],
                             start=True, stop=True)
            osb = fin_pool.tile([1, 1], F32)
            nc.vector.tensor_copy(osb[:], tot[0:1, 0:1])
            nc.sync.dma_start(out_d[:], osb[:])

    nc.compile()
    return nc


def kernel(seq, src_len, combinations):
    from concourse.bass_utils import run_bass_kernel_spmd

    plan, cores = pack(seq, src_len, combinations)
    nc = build_program(plan)
    in_maps = [{k: ci[k] for k in INPUT_NAMES} for ci in cores]
    res = run_bass_kernel_spmd(nc, in_maps, list(range(NCORES)))
    tot = np.float32(0.0)
    for c in range(NCORES):
        tot += np.float32(res.results[c]["out"][0, 0])
    n_pairs = np.asarray(combinations).shape[0]
    return np.float32(tot / np.float32(n_pairs))
